# revision 1
# baseline (speedup 1.0000x reference)
"""DIN-style sparse attention kernel for Trainium2, 8-core data parallel.

B=4096, T=200, D=64; MLP attention scores (256->80->40->1, sigmoid),
masked softmax over T, weighted sum of v. Pure data parallel: 512 batch
rows per core, same graph SPMD on 8 cores.

Math (per b, per t):
  info@W1 = k@(W1b-W1c) + (q*k)@W1d + q@(W1a+W1c)
          = k @ Weff_b + pre1_b,   Weff_b = (W1b-W1c) + diag(q_b)@W1d
  bf dropped (softmax shift-invariant). Mask applied additively as
  (mask-1)*30 to logits before exp. Softmax unnormalized:
  out_b = (sum_t e_t v_t) / (sum_t e_t).
"""

import sys
import numpy as np

sys.path.insert(0, "/opt/trn_rl_repo")

import ml_dtypes

BF16 = ml_dtypes.bfloat16

B_LOC = 512
T = 200
T1 = 128
T2 = T - T1
D = 64
H1 = 80
H2 = 40
GROUP = 8
NCORES = 8

_cache = {}


def build_graph():
    import concourse.bass as bass
    import concourse.mybir as mybir
    from concourse import tile
    from concourse.bacc import Bacc

    dt = mybir.dt
    AFT = mybir.ActivationFunctionType
    ALU = mybir.AluOpType

    f32 = dt.float32
    bf16 = dt.bfloat16
    i32 = dt.int32

    nc = Bacc()

    q_d = nc.dram_tensor("q", (B_LOC, D), f32, kind="ExternalInput")
    k_d = nc.dram_tensor("k", (B_LOC, T, D), f32, kind="ExternalInput")
    v_d = nc.dram_tensor("v", (B_LOC, T, D), f32, kind="ExternalInput")
    mask_d = nc.dram_tensor("mask", (B_LOC, T), i32, kind="ExternalInput")
    w1_d = nc.dram_tensor("W1", (4 * D, H1), f32, kind="ExternalInput")
    b1_d = nc.dram_tensor("b1", (H1,), f32, kind="ExternalInput")
    w2_d = nc.dram_tensor("W2", (H1, H2), f32, kind="ExternalInput")
    b2_d = nc.dram_tensor("b2", (H2,), f32, kind="ExternalInput")
    wf_d = nc.dram_tensor("Wf", (H2, 1), f32, kind="ExternalInput")
    bf_d = nc.dram_tensor("bf", (1,), f32, kind="ExternalInput")
    idb_d = nc.dram_tensor("ident_bf", (128, 128), bf16, kind="ExternalInput")
    idf_d = nc.dram_tensor("ident_f32", (128, 128), f32, kind="ExternalInput")
    ones_d = nc.dram_tensor("ones_f32", (128, 1), f32, kind="ExternalInput")
    out_d = nc.dram_tensor("out", (B_LOC, D), f32, kind="ExternalOutput")

    NG = B_LOC // GROUP      # 64 groups
    NPAIR = GROUP // 2       # 4

    with tile.TileContext(nc) as tc:
        with (
            tc.tile_pool(name="const", bufs=1) as cpool,
            tc.tile_pool(name="persist", bufs=1) as ppool,
        ):
            idb = cpool.tile([128, 128], bf16)
            nc.sync.dma_start(idb[:], idb_d[:])
            idf = cpool.tile([128, 128], f32)
            nc.sync.dma_start(idf[:], idf_d[:])
            ones = cpool.tile([128, 1], f32)
            nc.sync.dma_start(ones[:], ones_d[:])
            bfv = cpool.tile([1, 1], f32)
            nc.sync.dma_start(bfv[:], bf_d[:].rearrange("(a b) -> a b", b=1))

            # W1 slices double-stacked on partitions (rows 0:64 == rows 64:128)
            w1b_f = cpool.tile([128, H1], f32)
            w1c_f = cpool.tile([128, H1], f32)
            w1d_f = cpool.tile([128, H1], f32)

            def dup_load(dst, src):
                # duplicate the [64, H1] DRAM slice into both partition halves
                nc.sync.dma_start(dst[0:64, :], src)
                nc.sync.dma_start(dst[64:128, :], src)

            dup_load(w1b_f, w1_d[64:128, :])
            dup_load(w1c_f, w1_d[128:192, :])
            dup_load(w1d_f, w1_d[192:256, :])
            w1dd = cpool.tile([128, H1], bf16)
            nc.vector.tensor_copy(w1dd[:], w1d_f[:])
            w1a_f = cpool.tile([64, H1], f32)
            nc.sync.dma_start(w1a_f[:], w1_d[0:64, :])

            w2_bf = cpool.tile([H1, H2], bf16)
            nc.gpsimd.dma_start(w2_bf[:], w2_d[:])
            wf_bf = cpool.tile([H2, 1], bf16)
            nc.gpsimd.dma_start(wf_bf[:], wf_d[:])
            b1_sb = cpool.tile([H1, 1], f32)
            nc.sync.dma_start(b1_sb[:], b1_d[:].rearrange("(a b) -> a b", b=1))
            b2_sb = cpool.tile([H2, 1], f32)
            nc.sync.dma_start(b2_sb[:], b2_d[:].rearrange("(a b) -> a b", b=1))

            # funnel one operand of each 2-DMA-input op through a same-engine
            # copy: the TT ISA struct carries only one sync-wait slot.
            w1b_cp = cpool.tile([128, H1], f32)
            nc.vector.tensor_copy(w1b_cp[:], w1b_f[:])
            bm = cpool.tile([128, H1], bf16)          # Bm = W1b - W1c, stacked
            nc.vector.tensor_sub(bm[:], w1b_cp[:], w1c_f[:])
            w1a_cp = cpool.tile([64, H1], f32)
            nc.vector.tensor_copy(w1a_cp[:], w1a_f[:])
            apl = cpool.tile([64, H1], f32)           # A = W1a + W1c
            nc.vector.tensor_add(apl[:], w1a_cp[:], w1c_f[0:64, :])
            b1_cp = cpool.tile([H1, 1], f32)
            nc.vector.tensor_copy(b1_cp[:], b1_sb[:])

            qT_f = ppool.tile([64, B_LOC], f32)
            qT_both = ppool.tile([128, B_LOC], bf16)  # rows 64:128 duplicate rows 0:64
            pre1 = ppool.tile([H1, B_LOC], f32)
            m1T_a = ppool.tile([T1, B_LOC], f32)
            m1T_b = ppool.tile([T2, B_LOC], f32)

            with (
                tc.tile_pool(name="setup_sb", bufs=2) as spool,
                tc.tile_pool(name="setup_ps", bufs=2, space="PSUM") as spsum,
            ):
                for c in range(B_LOC // 128):
                    qc = spool.tile([128, D], f32, tag="qc")
                    nc.sync.dma_start(qc[:], q_d[c * 128:(c + 1) * 128, :])
                    qtp = spsum.tile([64, 128], f32, tag="qtp")
                    nc.tensor.transpose(qtp[:], qc[:], idf[:])
                    nc.vector.tensor_copy(qT_f[:, c * 128:(c + 1) * 128], qtp[:])
                    nc.vector.tensor_copy(
                        qT_both[0:64, c * 128:(c + 1) * 128], qtp[:])

                    mi = spool.tile([128, T], i32, tag="mi")
                    nc.sync.dma_start(mi[:], mask_d[c * 128:(c + 1) * 128, :])
                    m1 = spool.tile([128, T], f32, tag="m1")
                    nc.vector.tensor_scalar(m1[:], mi[:], 30.0, -30.0,
                                            ALU.mult, ALU.add)
                    mtp = spsum.tile([128, 128], f32, tag="mtp")
                    nc.tensor.transpose(mtp[:], m1[:, 0:T1], idf[:])
                    nc.vector.tensor_copy(m1T_a[:, c * 128:(c + 1) * 128], mtp[:])
                    mtp2 = spsum.tile([T2, 128], f32, tag="mtp2")
                    nc.tensor.transpose(mtp2[:], m1[:, T1:T], idf[0:128, 0:128])
                    nc.vector.tensor_copy(m1T_b[:, c * 128:(c + 1) * 128], mtp2[:])

                pre1_ps = spsum.tile([H1, B_LOC], f32, tag="pre1ps")
                nc.tensor.matmul(pre1_ps[:], apl[:], qT_f[:])
                nc.vector.tensor_scalar_add(pre1[:], pre1_ps[:], b1_cp[:])
                # duplicate qT rows into partitions 64:128 (cross-partition: DMA)
                nc.sync.dma_start(qT_both[64:128, :], qT_both[0:64, :])

            # ------------------------- main loop -------------------------
            with (
                tc.tile_pool(name="kin", bufs=3) as kin,
                tc.tile_pool(name="vin", bufs=3) as vin,
                tc.tile_pool(name="ktps", bufs=2, space="PSUM") as ktps,
                tc.tile_pool(name="ktsb", bufs=3) as ktsb,
                tc.tile_pool(name="weff", bufs=4) as weffp,
                tc.tile_pool(name="h1ps", bufs=2, space="PSUM") as h1psp,
                tc.tile_pool(name="h1sb", bufs=3) as h1sbp,
                tc.tile_pool(name="h2ps", bufs=1, space="PSUM") as h2psp,
                tc.tile_pool(name="h2sb", bufs=3) as h2sbp,
                tc.tile_pool(name="lgps", bufs=1, space="PSUM") as lgpsp,
                tc.tile_pool(name="expw", bufs=2) as expwp,
                tc.tile_pool(name="ndps", bufs=1, space="PSUM") as ndpsp,
                tc.tile_pool(name="ftps", bufs=1, space="PSUM") as ftpsp,
                tc.tile_pool(name="fin", bufs=2) as finp,
                tc.tile_pool(name="outp", bufs=2) as outp,
            ):
                out_sb = None
                for g in range(NG):
                    b0 = g * GROUP
                    k1 = kin.tile([T1, GROUP, D], bf16, tag="k1")
                    k2 = kin.tile([T2, GROUP, D], bf16, tag="k2")
                    nc.gpsimd.dma_start(
                        k1[:], k_d[b0:b0 + GROUP, 0:T1, :].rearrange("b t d -> t b d"))
                    nc.gpsimd.dma_start(
                        k2[:], k_d[b0:b0 + GROUP, T1:T, :].rearrange("b t d -> t b d"))
                    v1 = vin.tile([T1, GROUP, D], f32, tag="v1")
                    v2 = vin.tile([T2, GROUP, D], f32, tag="v2")
                    nc.sync.dma_start(
                        v1[:], v_d[b0:b0 + GROUP, 0:T1, :].rearrange("b t d -> t b d"))
                    nc.sync.dma_start(
                        v2[:], v_d[b0:b0 + GROUP, T1:T, :].rearrange("b t d -> t b d"))

                    lg_ps = lgpsp.tile([128, 2 * GROUP], f32, tag="lg")
                    # rows 72:128 of odd (t-chunk-2) columns are never written
                    # by the L3 matmuls; zero them so exp reads defined data
                    nc.vector.memset(
                        lg_ps[64:128, :].rearrange("p (a two) -> p two a", two=2)[:, 1, :],
                        0.0)

                    for p in range(NPAIR):
                        bp = b0 + 2 * p
                        ktp = ktps.tile([128, T], bf16, tag="ktp")
                        nc.tensor.transpose(
                            ktp[:, 0:T1],
                            k1[:, 2 * p:2 * p + 2, :].rearrange("t b d -> t (b d)"),
                            idb[:])
                        nc.tensor.transpose(
                            ktp[:, T1:T],
                            k2[:, 2 * p:2 * p + 2, :].rearrange("t b d -> t (b d)"),
                            idb[0:T2, 0:T2])
                        kts = ktsb.tile([128, T], bf16, tag="kts")
                        nc.vector.tensor_copy(kts[:], ktp[:])

                        wef = weffp.tile([128, H1], bf16, tag="wef")
                        nc.vector.scalar_tensor_tensor(
                            wef[0:64, :], w1dd[0:64, :],
                            qT_both[0:64, bp:bp + 1], bm[0:64, :],
                            ALU.mult, ALU.add)
                        nc.vector.scalar_tensor_tensor(
                            wef[64:128, :], w1dd[64:128, :],
                            qT_both[64:128, bp + 1:bp + 2], bm[64:128, :],
                            ALU.mult, ALU.add)

                        h1_ps = h1psp.tile([H1, 2 * T], f32, tag="h1")
                        h1_sb = h1sbp.tile([H1, 2 * T], bf16, tag="h1s")
                        for hh in range(2):
                            bb = bp + hh
                            nc.tensor.matmul(
                                h1_ps[:, hh * T:(hh + 1) * T],
                                wef[hh * 64:(hh + 1) * 64, :],
                                kts[hh * 64:(hh + 1) * 64, :])
                            nc.scalar.activation(
                                h1_sb[:, hh * T:(hh + 1) * T],
                                h1_ps[:, hh * T:(hh + 1) * T],
                                AFT.Sigmoid, bias=pre1[:, bb:bb + 1])

                        h2_ps = h2psp.tile([H2, 2 * T], f32, tag="h2")
                        nc.tensor.matmul(h2_ps[:], w2_bf[:], h1_sb[:])
                        h2_sb = h2sbp.tile([H2, 2 * T], bf16, tag="h2s")
                        nc.scalar.activation(h2_sb[:], h2_ps[:], AFT.Sigmoid,
                                             bias=b2_sb[:])

                        for hh in range(2):
                            j = 2 * p + hh        # b index within group, 0..7
                            nc.tensor.matmul(
                                lg_ps[0:T1, 2 * j:2 * j + 1],
                                h2_sb[:, hh * T:hh * T + T1], wf_bf[:])
                            nc.tensor.matmul(
                                lg_ps[0:T2, 2 * j + 1:2 * j + 2],
                                h2_sb[:, hh * T + T1:(hh + 1) * T], wf_bf[:])

                    # mask add + exp for the group (8 b's, 16 cols)
                    lgr = lg_ps[:].rearrange("p (a two) -> p two a", two=2)
                    nc.vector.tensor_add(
                        lgr[:, 0, :], lgr[:, 0, :], m1T_a[:, b0:b0 + GROUP])
                    lgr2 = lg_ps[0:T2, :].rearrange("p (a two) -> p two a", two=2)
                    nc.vector.tensor_add(
                        lgr2[:, 1, :], lgr2[:, 1, :], m1T_b[:, b0:b0 + GROUP])
                    ew = expwp.tile([128, 2 * GROUP], f32, tag="ew")
                    nc.scalar.activation(ew[:], lg_ps[:], AFT.Exp)

                    # denominator and numerator into one [65, 8] psum tile
                    nd_ps = ndpsp.tile([65, GROUP], f32, tag="nd")
                    ewr = ew[:].rearrange("p (a two) -> p two a", two=2)
                    nc.tensor.matmul(nd_ps[64:65, :], ones[:, 0:1], ewr[:, 0, :],
                                     start=True, stop=False)
                    nc.tensor.matmul(nd_ps[64:65, :], ones[0:T2, 0:1],
                                     ewr[0:T2, 1, :], start=False, stop=True)
                    for j in range(GROUP):
                        nc.tensor.matmul(
                            nd_ps[0:64, j:j + 1],
                            v1[:, j, :], ew[:, 2 * j:2 * j + 1],
                            start=True, stop=False)
                        nc.tensor.matmul(
                            nd_ps[0:64, j:j + 1],
                            v2[:, j, :], ew[0:T2, 2 * j + 1:2 * j + 2],
                            start=False, stop=True)

                    # finalize group: divide and transpose to natural layout
                    fin = finp.tile([65, GROUP], f32, tag="fin")
                    nc.vector.tensor_copy(fin[:], nd_ps[:])
                    ft_ps = ftpsp.tile([GROUP, 65], f32, tag="ft")
                    nc.tensor.transpose(ft_ps[:], fin[:], idf[0:65, 0:65])
                    rden = finp.tile([GROUP, 1], f32, tag="rden")
                    nc.vector.reciprocal(rden[:], ft_ps[:, 64:65])
                    if g % 8 == 0:
                        out_sb = outp.tile([GROUP, 8, D], f32, tag="osb")
                    nc.vector.tensor_scalar_mul(
                        out_sb[:, g % 8, :], ft_ps[:, 0:64], rden[:])
                    if g % 8 == 7:
                        nc.sync.dma_start(
                            out_d[(g - 7) * GROUP:(g + 1) * GROUP, :]
                            .rearrange("(gg p) d -> p gg d", p=GROUP),
                            out_sb[:])

    nc.finalize()
    return nc


def _axon_claim_fixup(timeout_s=45):
    """Work around stale axon terminal session claims: rebuild the backend
    factory with claim_timeout_s so a dead session's claim is stolen instead
    of waited on forever. No-op if the backend is already initialized."""
    try:
        import functools
        import uuid

        from jax._src import xla_bridge as xb
        from jax._src.lib import xla_client

        if "axon" in getattr(xb, "_backends", {}):
            return
        reg = xb._backend_factories.get("axon")
        if reg is None:
            return
        opts = dict(reg.factory.keywords.get("options", {}))
        if not opts or "claim_timeout_s" in opts:
            return
        opts["claim_timeout_s"] = timeout_s
        opts["session_id"] = str(uuid.uuid4())
        xb._backend_factories["axon"] = xb.BackendRegistration(
            factory=functools.partial(
                xla_client.make_c_api_client, "axon", opts, None),
            priority=reg.priority, fail_quietly=False, experimental=True,
            c_api=reg.c_api)
    except Exception:
        pass


def _get_graph():
    if "nc" not in _cache:
        _cache["nc"] = build_graph()
    return _cache["nc"]


def kernel(**inputs):
    _axon_claim_fixup()
    from concourse import bass_utils

    nc = _get_graph()

    q = np.asarray(inputs["q"], np.float32)
    k = np.asarray(inputs["k"], np.float32)
    v = np.asarray(inputs["v"], np.float32)
    mask = np.asarray(inputs["mask"], np.int32)
    consts = {
        "W1": np.asarray(inputs["W1"], np.float32),
        "b1": np.asarray(inputs["b1"], np.float32),
        "W2": np.asarray(inputs["W2"], np.float32),
        "b2": np.asarray(inputs["b2"], np.float32),
        "Wf": np.asarray(inputs["Wf"], np.float32),
        "bf": np.asarray(inputs["bf"], np.float32),
        "ident_bf": np.eye(128, dtype=BF16),
        "ident_f32": np.eye(128, dtype=np.float32),
        "ones_f32": np.ones((128, 1), np.float32),
    }
    in_maps = []
    for i in range(NCORES):
        s = slice(i * B_LOC, (i + 1) * B_LOC)
        in_maps.append({"q": q[s], "k": k[s], "v": v[s], "mask": mask[s], **consts})

    res = bass_utils.run_bass_kernel_spmd(nc, in_maps, core_ids=list(range(NCORES)))
    outs = [res.results[i]["out"] for i in range(NCORES)]
    return np.concatenate(outs, axis=0)

